# revision 1
# baseline (speedup 1.0000x reference)
"""Attention-Augmented Conv2D fused Bass kernel for 8 trn2 NeuronCores.

Problem (hardcoded): x [4,64,32,32], NH=8, DK=DV=64, FILTERS=128 -> out [4,128,32,32].
Sharding: core c -> batch b=c//2, head-group g=c%2 (heads 4g..4g+4).
Each core produces:
  o_conv [64,512]  : conv1x1 output for its batch, positions [512g, 512g+512)
  o_attn [64,1024] : partial attn-out conv over its 4 heads (bias only on g==0)
Host gather: conv halves concatenated, attn partials summed per batch.

All projections fold their bias via a ones-row appended to x (x_aug [65,1024],
uploaded in bf16). Relative-position logits fold into the single logits matmul
with K-dim 72:
    KA_i = [D_w (32, k%32 indicator) ; D_h (32, k//32 indicator) ; K_i (8)]
    QA_i = [patwT (32) ; pathT (32) ; Q_i (8)]
patwT/pathT are computed with 32+32 tiny matmuls per head against host-built
shifted (Toeplitz) tables wps[c, 2i+t, 32*blk+j] = wpat_t_i[c, 31+j-blk], with
x as the moving tensor -- same PE column count as a plain projection, and no
DMA scratch roundtrip. A DVE copy unscrambles patw from its y-major (q'=32y+u)
PSUM layout into q-major bf16 QA rows.

Softmax skips max-subtraction (logits are O(few)); the denominator comes from a
ones-column in the V projection, so the PV matmul also produces the softmax
denominator (pv row 32i per head). The epilogue is per-head and overlaps the
next head's main loop: copy pv rows to SBUF (sole pv reader, so the next head's
PSUM writes only wait ~1.3us) -> reciprocal (DVE) -> partition_broadcast
(GPSIMD) -> pv*rp (DVE, bf16) -> wattn^T @ attn accumulated on PE one head
later. Head 3's epilogue runs in the tail, split into qc halves, phase-major.
conv1x1 also runs in the tail shadow. A short PE warmup at t~0 starts the
p-state ramp clock so the real projections run at full clock.
"""
import sys
import numpy as np

sys.path.insert(0, '/opt/trn_rl_repo')

NH, DK, DV, FILTERS = 8, 64, 64, 128
B, C, H, W = 4, 64, 32, 32
HW = H * W
dkh = DK // NH
SCALE = dkh ** -0.5
N_CORES = 8


def _build_bass():
    import concourse.bass as bass
    import concourse.bacc as bacc
    import concourse.mybir as mybir
    import concourse.tile as tile

    f32 = mybir.dt.float32
    bf16 = mybir.dt.bfloat16
    AF = mybir.ActivationFunctionType

    nc = bacc.Bacc()

    xbf = nc.dram_tensor("xbf", [65, HW], bf16, kind="ExternalInput")
    xc = nc.dram_tensor("xc", [65, 512], bf16, kind="ExternalInput")
    wcat = nc.dram_tensor("wcat", [65, 164], bf16, kind="ExternalInput")
    wps = nc.dram_tensor("wps", [65, 8, HW], bf16, kind="ExternalInput")
    wtail = nc.dram_tensor("wtail", [128, 64], bf16, kind="ExternalInput")
    wt9 = nc.dram_tensor("wt9", [9, 256], bf16, kind="ExternalInput")
    dconst = nc.dram_tensor("dconst", [64, HW], bf16, kind="ExternalInput")
    o_conv = nc.dram_tensor("o_conv", [64, 512], f32, kind="ExternalOutput")
    o_attn = nc.dram_tensor("o_attn", [64, HW], f32, kind="ExternalOutput")

    with tile.TileContext(nc) as tc:
        with (
            tc.tile_pool(name="const", bufs=1) as constp,
            tc.tile_pool(name="kaqa", bufs=4) as kaqap,
            tc.tile_pool(name="pt", bufs=5) as ptp,
            tc.tile_pool(name="ep", bufs=3) as epp,
            tc.tile_pool(name="outp", bufs=1) as outp,
            tc.tile_pool(name="ps_lg", bufs=2, space="PSUM") as ps_lg,
            tc.tile_pool(name="ps_pf", bufs=1, space="PSUM") as ps_pf,
            tc.tile_pool(name="ps_pv", bufs=1, space="PSUM") as ps_pv,
        ):
            # ---- input DMAs ----
            x_sb = constp.tile([65, HW], bf16, tag="x")
            for qc in range(2):
                nc.sync.dma_start(out=x_sb[:, 512 * qc:512 * qc + 512],
                                  in_=xbf[:, 512 * qc:512 * qc + 512])
            wcat_sb = constp.tile([65, 164], bf16, tag="wcat")
            nc.scalar.dma_start(out=wcat_sb, in_=wcat[:, :])
            # shifted rel tables: wps[c, 2i+t, 32*blk + j] = wpat_t_i[c, 31+j-blk]
            wps_sb = constp.tile([65, 8, HW], bf16, tag="wps")
            nc.scalar.dma_start(out=wps_sb[:, 0:2, :], in_=wps[:, 0:2, :])
            nc.sync.dma_start(out=wps_sb[:, 2:8, :], in_=wps[:, 2:8, :])
            xc_sb = constp.tile([65, 512], bf16, tag="xc")
            wqk_sb = wcat_sb[:, 0:64]
            wva_sb = wcat_sb[:, 64:100]
            wconv_sb = wcat_sb[:, 100:164]
            wattn_sb = constp.tile([128, 64], bf16, tag="wtail")
            wt9_sb = constp.tile([9, 256], bf16, tag="wt9")

            ka, qa = [], []
            for i in range(4):
                ka_i = kaqap.tile([72, HW], bf16, tag="ka", name=f"ka{i}")
                qa_i = kaqap.tile([72, HW], bf16, tag="qa", name=f"qa{i}")
                ka.append(ka_i)
                qa.append(qa_i)
            nc.gpsimd.dma_start(out=ka[0][0:64, :], in_=dconst[:, :])

            # ---- PE warmup: start the p-state ramp clock early ----
            warm_s = constp.tile([1, 16], f32, tag="warms")
            warm_m = constp.tile([1, 128], f32, tag="warmm")
            nc.vector.memset(warm_s, 0.0)
            nc.vector.memset(warm_m, 0.0)
            warm_ps = ps_lg.tile([16, 128], f32, tag="g")
            for _ in range(4):
                nc.tensor.matmul(warm_ps[:, :], warm_s[:, 0:16], warm_m[:, :])
            # preload the activation table while Act waits for inputs
            warm_a = constp.tile([1, 16], f32, tag="warma")
            nc.scalar.activation(warm_a, warm_s[:, :], AF.Exp)

            # ---- qk projection ----
            qk_ps = ps_pv.tile([64, HW], f32, tag="v")
            for qc in range(2):
                nc.tensor.matmul(qk_ps[:, 512 * qc:512 * qc + 512],
                                 wqk_sb, x_sb[:, 512 * qc:512 * qc + 512])

            xp = x_sb.rearrange("c (u y) -> c y u", y=32)
            rel_pss = []

            def emit_rel_mms(i):
                # patwT/pathT via 32+32 shifted-table matmuls, no DMA roundtrip.
                # rows 0..32: patw in y-major (q' = 32y+u) layout; 32..64: path.
                rel_ps = ps_pf.tile([64, HW], f32, tag="f", name=f"rel{i}")
                rel_pss.append(rel_ps)
                for y in range(32):
                    nc.tensor.matmul(
                        rel_ps[0:32, 32 * y:32 * y + 32],
                        wps_sb[:, 2 * i, 32 * y:32 * y + 32],
                        xp[:, y, :])
                for u in range(32):
                    nc.tensor.matmul(
                        rel_ps[32:64, 32 * u:32 * u + 32],
                        wps_sb[:, 2 * i + 1, 32 * u:32 * u + 32],
                        x_sb[:, 32 * u:32 * u + 32],
                        tile_position=(0, 32))

            def emit_rel_copies(i):
                # DVE: unscramble patw q'->q + convert; path is already q-major
                # (head 0's path copy runs on the idle Act engine in parallel)
                nc.vector.tensor_copy(
                    out=qa[i][0:32, :].rearrange("p (u y) -> p u y", y=32),
                    in_=rel_pss[i][0:32, :].rearrange("p (y u) -> p u y", u=32))
                if i == 0:
                    nc.scalar.activation(qa[i][32:64, :],
                                         rel_pss[i][32:64, :], AF.Copy)
                else:
                    nc.vector.tensor_copy(out=qa[i][32:64, :],
                                          in_=rel_pss[i][32:64, :])

            def emit_qk_rows(i):
                nc.gpsimd.dma_start(out=ka[i][64:72, :],
                                    in_=qk_bf[32 + 8 * i:40 + 8 * i, :])
                nc.gpsimd.dma_start(out=qa[i][64:72, :],
                                    in_=qk_bf[8 * i:8 * i + 8, :])

            # qkc early on Act, then head-0 critical chain
            qk_bf = constp.tile([64, HW], bf16, tag="qk")
            nc.scalar.activation(qk_bf, qk_ps[:, :], AF.Copy)
            emit_qk_rows(0)
            emit_rel_mms(0)
            emit_rel_copies(0)
            emit_qk_rows(1)
            nc.gpsimd.dma_start(out=ka[1][0:64, :], in_=ka[0][0:64, :])
            # V^T projection (needed at pv(0,0), shortly after lg(0,0))
            vt_ps = ps_lg.tile([128, 8, 36], f32, tag="g")
            for kt in range(8):
                nc.tensor.matmul(vt_ps[:, kt, :],
                                 x_sb[:, 128 * kt:128 * kt + 128], wva_sb)
            vt_sb = constp.tile([128, 8, 36], bf16, tag="vt")
            nc.scalar.activation(vt_sb, vt_ps[:, :, :], AF.Copy)
            nc.gpsimd.dma_start(out=wattn_sb, in_=wtail[:, :])
            nc.gpsimd.dma_start(out=wt9_sb, in_=wt9[:, :])

            # ---- main attention loop (heads 2-3 projections interleaved) ----
            pv_ps = ps_pv.tile([128, HW], f32, tag="v")
            attn_n = outp.tile([128, HW], bf16, tag="attn")
            oat_sb = outp.tile([64, HW], f32, tag="oat")
            conv_sb = outp.tile([64, 512], f32, tag="oconv")
            oat_ps = None
            seq = [(i, kt) for i in range(4) for kt in range(8)]

            def emit_lg(i, kt):
                lg_ps = ps_lg.tile([128, HW], f32, tag="g")
                for qc in range(2):
                    nc.tensor.matmul(
                        lg_ps[:, 512 * qc:512 * qc + 512],
                        ka[i][:, 128 * kt:128 * kt + 128],
                        qa[i][:, 512 * qc:512 * qc + 512])
                return lg_ps

            attn_i = [None] * 4

            def emit_norm(i):
                # DVE/GPSIMD only -- never blocks the PE stream. Heads 0-2
                # first copy their pv rows to SBUF (the only pv reader, so the
                # next head's pv writes only wait ~1.3us, not the full chain).
                at = epp.tile([9, HW], bf16, tag="attn", name=f"attn{i}")
                attn_i[i] = at
                rp = epp.tile([1, HW], f32, tag="rp")
                rpb = epp.tile([9, HW], f32, tag="rpb")
                if i < 3:
                    pvc = epp.tile([9, HW], f32, tag="pvc")
                    nc.vector.tensor_copy(out=pvc, in_=pv_ps[32 * i:32 * i + 9, :])
                    nc.vector.reciprocal(out=rp, in_=pvc[0:1, :])
                    nc.gpsimd.partition_broadcast(rpb[0:9, :], rp[0:1, :])
                    nc.vector.tensor_mul(at[0:9, :], pvc[0:9, :], rpb[0:9, :])
                else:
                    # phase-major qc-split: DVE runs recip-a, recip-b, mul-a,
                    # mul-b back-to-back; Pool bcasts pipeline between them
                    sls = [slice(0, 512), slice(512, 1024)]
                    rphs, rpbhs = [], []
                    for qc in range(2):
                        rph = epp.tile([1, 512], f32, tag=f"rp3{qc}",
                                       name=f"rp3{qc}")
                        rphs.append(rph)
                        nc.vector.reciprocal(out=rph, in_=pv_ps[96:97, sls[qc]])
                    for qc in range(2):
                        rpbh = epp.tile([9, 512], f32, tag=f"rpb3{qc}",
                                        name=f"rpb3{qc}")
                        rpbhs.append(rpbh)
                        nc.gpsimd.partition_broadcast(rpbh[0:9, :],
                                                      rphs[qc][0:1, :])
                    for qc in range(2):
                        nc.vector.tensor_mul(attn_n[96:105, sls[qc]],
                                             pv_ps[96:105, sls[qc]],
                                             rpbhs[qc][0:9, :])
                    for qc in range(2):
                        nc.tensor.matmul(
                            oat_ps[0:64, sls[qc]], wattn_sb[96:105, :],
                            attn_n[96:105, sls[qc]], start=False, stop=True,
                            tile_position=(96, 0))
                    for qc in range(2):
                        if qc == 0:
                            nc.scalar.activation(oat_sb[:, sls[qc]],
                                                 oat_ps[0:64, sls[qc]], AF.Copy)
                        else:
                            nc.vector.tensor_copy(out=oat_sb[:, sls[qc]],
                                                  in_=oat_ps[0:64, sls[qc]])
                        nc.sync.dma_start(out=o_attn[:, sls[qc]],
                                          in_=oat_sb[:, sls[qc]])

            def emit_oat(i, stop):
                for qc in range(2):
                    if i < 3:
                        nc.tensor.matmul(
                            oat_ps[0:64, 512 * qc:512 * qc + 512],
                            wt9_sb[0:9, 64 * i:64 * i + 64],
                            attn_i[i][0:9, 512 * qc:512 * qc + 512],
                            start=(i == 0), stop=stop)
                    else:
                        nc.tensor.matmul(
                            oat_ps[0:64, 512 * qc:512 * qc + 512],
                            wattn_sb[96:105, :],
                            attn_n[96:105, 512 * qc:512 * qc + 512],
                            start=False, stop=stop,
                            tile_position=(96, 0))

            pts = {}

            def emit_pv(i, kt):
                pt = pts.pop((i, kt))
                for qc in range(2):
                    nc.tensor.matmul(
                        pv_ps[32 * i:32 * i + 9, 512 * qc:512 * qc + 512],
                        vt_sb[:, kt, 9 * i:9 * i + 9],
                        pt[:, 512 * qc:512 * qc + 512],
                        start=(kt == 0), stop=(kt == 7),
                        tile_position=(0, 32 * i))
                if kt == 7 and i >= 1:
                    emit_oat(i - 1, stop=False)
                if kt == 7 and i < 3:
                    emit_norm(i)

            lg_tiles = {seq[0]: emit_lg(*seq[0])}
            for j, (i, kt) in enumerate(seq):
                if j + 1 < len(seq):
                    lg_tiles[seq[j + 1]] = emit_lg(*seq[j + 1])
                lg_ps = lg_tiles.pop((i, kt))
                pt = ptp.tile([128, HW], bf16, tag="pt")
                nc.scalar.activation(pt, lg_ps[:, :], AF.Exp)
                pts[(i, kt)] = pt
                # pv matmuls run two iterations behind their exp so lg(j)
                # never queues behind a pt-gated pv in the PE stream
                if j >= 2:
                    emit_pv(*seq[j - 2])
                if (i, kt) == (0, 2):
                    emit_rel_mms(1)
                    emit_rel_copies(1)
                if (i, kt) == (0, 5):
                    emit_rel_mms(2)
                    emit_rel_copies(2)
                    nc.gpsimd.dma_start(out=ka[2][0:64, :], in_=ka[0][0:64, :])
                    emit_qk_rows(2)
                if (i, kt) == (1, 3):
                    emit_rel_mms(3)
                    emit_rel_copies(3)
                    nc.gpsimd.dma_start(out=ka[3][0:64, :], in_=ka[0][0:64, :])
                    emit_qk_rows(3)
                if (i, kt) == (1, 0):
                    oat_ps = ps_pf.tile([64, HW], f32, tag="f")
                if (i, kt) == (1, 6):
                    nc.gpsimd.dma_start(out=xc_sb, in_=xc[:, :])

            # drain the two deferred pv iterations ((3,6) and (3,7))
            emit_pv(*seq[30])
            emit_pv(*seq[31])

            # ---- tail: conv (PE free now) + head-3 epilogue + outputs ----
            conv_ps = ps_lg.tile([64, 512], f32, tag="g")
            nc.tensor.matmul(conv_ps[:, :], wconv_sb, xc_sb)
            emit_norm(3)
            nc.scalar.activation(conv_sb, conv_ps[:, :], AF.Copy)
            nc.sync.dma_start(out=o_conv[:, :], in_=conv_sb)

    nc.compile()
    return nc


def _host_prep(inputs):
    import ml_dtypes
    bf = ml_dtypes.bfloat16
    x = np.ascontiguousarray(inputs['x'], np.float32)
    w_qkv = np.ascontiguousarray(inputs['w_qkv'].reshape(2 * DK + DV, C), np.float32)
    b_qkv = np.ascontiguousarray(inputs['b_qkv'], np.float32)
    w_conv = np.ascontiguousarray(inputs['w_conv'].reshape(FILTERS - DV, C), np.float32)
    b_conv = np.ascontiguousarray(inputs['b_conv'], np.float32)
    w_attn = np.ascontiguousarray(inputs['w_attn'].reshape(DV, DV), np.float32)
    b_attn = np.ascontiguousarray(inputs['b_attn'], np.float32)
    rel_h = np.ascontiguousarray(inputs['key_rel_h'], np.float32)  # [63, 8]
    rel_w = np.ascontiguousarray(inputs['key_rel_w'], np.float32)  # [63, 8]

    kk = np.arange(HW)
    DCmat = np.zeros((64, HW), np.float32)
    DCmat[:32] = (kk[None, :] % 32 == np.arange(32)[:, None])
    DCmat[32:] = (kk[None, :] // 32 == np.arange(32)[:, None])
    DCmat = DCmat.astype(bf)

    wconv_aug = np.concatenate([w_conv, b_conv[:, None]], 1).T  # [65, 64]

    in_maps = []
    for c in range(N_CORES):
        b, g = c // 2, c % 2
        heads = [4 * g + i for i in range(4)]
        x_aug = np.concatenate([x[b].reshape(C, HW),
                                np.ones((1, HW), np.float32)], 0)
        wq = w_qkv[32 * g:32 * g + 32] * SCALE
        bq = b_qkv[32 * g:32 * g + 32] * SCALE
        wk = w_qkv[64 + 32 * g:64 + 32 * g + 32]
        bk = b_qkv[64 + 32 * g:64 + 32 * g + 32]
        wqk_aug = np.concatenate(
            [np.concatenate([wq, wk], 0),
             np.concatenate([bq, bk], 0)[:, None]], 1).T  # [65, 64]
        wva_m = np.zeros((65, 36), np.float32)
        wpat_w = np.zeros((65, 4, 63), np.float32)
        wpat_h = np.zeros((65, 4, 63), np.float32)
        for i, h in enumerate(heads):
            wv = w_qkv[128 + 8 * h:128 + 8 * h + 8]
            bv = b_qkv[128 + 8 * h:128 + 8 * h + 8]
            wva_m[64, 9 * i] = 1.0
            wva_m[:64, 9 * i + 1:9 * i + 9] = wv.T
            wva_m[64, 9 * i + 1:9 * i + 9] = bv
            wq_h = w_qkv[8 * h:8 * h + 8] * SCALE
            bq_h = b_qkv[8 * h:8 * h + 8] * SCALE
            wpat_w[:64, i, :] = (rel_w @ wq_h).T
            wpat_w[64, i, :] = rel_w @ bq_h
            wpat_h[:64, i, :] = (rel_h @ wq_h).T
            wpat_h[64, i, :] = rel_h @ bq_h
        # shifted tables: wps[c, t*4+i, 32*blk + j] = wpat_t[c, i, 31 + j - blk]
        wps_m = np.zeros((65, 8, 32, 32), np.float32)
        blk = np.arange(32)[:, None]
        jj = np.arange(32)[None, :]
        sel = 31 + jj - blk  # [32 blk, 32 j] in [0, 62]
        for i in range(4):
            wps_m[:, 2 * i] = wpat_w[:, i, :][:, sel]
            wps_m[:, 2 * i + 1] = wpat_h[:, i, :][:, sel]
        wattn_aug = np.zeros((128, 64), np.float32)
        wt9_m = np.zeros((9, 4, 64), np.float32)
        for i, h in enumerate(heads):
            wattn_aug[32 * i + 1:32 * i + 9] = w_attn[:, 8 * h:8 * h + 8].T
            wt9_m[1:9, i, :] = w_attn[:, 8 * h:8 * h + 8].T
        if g == 0:
            wattn_aug[0] += b_attn
            wt9_m[0, 0, :] += b_attn
        wcat = np.concatenate([wqk_aug, wva_m, wconv_aug], 1)  # [65, 164]
        in_maps.append({
            'xbf': np.ascontiguousarray(x_aug.astype(bf)),
            'xc': np.ascontiguousarray(
                x_aug[:, 512 * g:512 * g + 512].astype(bf)),
            'wcat': np.ascontiguousarray(wcat.astype(bf)),
            'wps': np.ascontiguousarray(
                wps_m.reshape(65, 8, 1024).astype(bf)),
            'wtail': np.ascontiguousarray(wattn_aug.astype(bf)),
            'wt9': np.ascontiguousarray(wt9_m.reshape(9, 256).astype(bf)),
            'dconst': DCmat,
        })
    return in_maps


_CACHED = {}


def kernel(**inputs):
    from concourse.bass_utils import run_bass_kernel_spmd
    if 'nc' not in _CACHED:
        _CACHED['nc'] = _build_bass()
    nc = _CACHED['nc']
    in_maps = _host_prep(inputs)
    res = run_bass_kernel_spmd(nc, in_maps, core_ids=list(range(N_CORES)))
    out = np.zeros((B, FILTERS, HW), np.float32)
    for c in range(N_CORES):
        b, g = c // 2, c % 2
        out[b, :64, 512 * g:512 * g + 512] = res.results[c]['o_conv']
        out[b, 64:] += res.results[c]['o_attn']
    return out.reshape(B, FILTERS, H, W)

